# revision 14
# baseline (speedup 1.0000x reference)
"""TRN2 Bass kernel for nn_Knowledge_Base (retrieval_knn).

reference:
    proj = word_output @ W.T + b            # [B,S,H]
    dis  = -sqrt(sum((proj[...,None,:] - op_emb)**2, -1))   # [B,S,O]
    prob = softmax(dis, -1); prob[prob < 0.3] = 0

Strategy (8 cores, data-parallel over the 8192 tokens, 1024/core):
  d2(o) = ||W x||^2 + m_o . x + n_o   with host-precomputed (fp64)
      m_o = 2 W^T (b - e_o)  in R^768,   n_o = ||b - e_o||^2.
  This decouples per-o accuracy (exact m/n + 3-term bf16-split x.m
  matmul) from ||Wx||^2, whose error is shared across all 32 ops and
  attenuated ~60x by the sqrt curvature, so a 2-pass bf16 proj
  (Wh@xh + Wh@xl) suffices; the residual is an algebraic omission
  (missing Wl@x), identical on HW and in simulation (~5e-6 prob error
  at the tightest threshold margin).
  The whole d2 accumulates in ONE [o, t] PSUM tile: 18 lin passes,
  then per h-chunk the squared proj (ACT Square -> bf16 hi/lo planes)
  enters via ones-stationary matmuls (broadcasts ||Wx||^2 over o), and
  n_o enters as one K=3 matmul of its 3-way bf16 split against ones.
  Transpose d2 to [t, o] on the PE, then a 3-engine tail per 128-token
  chunk: ACT ln/exp for s0 ~ sqrt(d2), one DVE Newton step
  (bit-exact reciprocal) to kill the ~1e-4 act-table error, ACT
  exp(-u/2), DVE normalize, GPSIMD threshold.  All ACT funcs live in
  the one covering act table set (no mid-kernel table reloads).
  Host pre/post: x pre-split bf16 hi/lo, pre-tiled so every DMA is
  contiguous per partition (x tile DMAs split in halves so the PE
  starts after the first half); output written in on-chip layout and
  unpermuted on host.
"""
import sys
sys.path.insert(0, "/opt/trn_rl_repo")
import numpy as np
import ml_dtypes

import concourse.bacc as bacc
import concourse.tile as tile
from concourse import mybir
from concourse import bass_utils

BF = ml_dtypes.bfloat16
P = 128
B, S, E, H, O = 4, 2048, 768, 512, 32
NCORES = 8
TOK = B * S                  # 8192
TPC = TOK // NCORES          # 1024 tokens per core
TT = 512                     # t-tile size
NTT = TPC // TT              # 2 t-tiles per core
EC = E // P                  # 6 e-chunks
HC = H // P                  # 4 h-chunks
CC = TT // P                 # 4 c-chunks (token blocks) per t-tile
EH = 3                       # e-chunks in first DMA half
THRESH = 0.3

_CACHE = {}


def _split(a):
    hi = a.astype(BF)
    lo = (a.astype(np.float32) - hi.astype(np.float32)).astype(BF)
    return hi, lo


def _build():
    nc = bacc.Bacc("TRN2", target_bir_lowering=False, debug=False,
                   num_devices=NCORES)
    dt = mybir.dt
    xh_d = nc.dram_tensor("xh", [NTT, P, EC, TT], dt.bfloat16,
                          kind="ExternalInput").ap()
    xl_d = nc.dram_tensor("xl", [NTT, P, EC, TT], dt.bfloat16,
                          kind="ExternalInput").ap()
    wh_d = nc.dram_tensor("wh", [P, EC, H], dt.bfloat16,
                          kind="ExternalInput").ap()
    mpk_d = nc.dram_tensor("mpk", [P, EC, 2 * O], dt.bfloat16,
                           kind="ExternalInput").ap()
    # n3: [3, O] bf16 three-way split of ||b - e_o||^2
    n3_d = nc.dram_tensor("n3", [3, O], dt.bfloat16,
                          kind="ExternalInput").ap()
    out_d = nc.dram_tensor("out", [P, NTT, CC, O], dt.float32,
                           kind="ExternalOutput").ap()

    with tile.TileContext(nc) as tc:
        with tc.tile_pool(name="consts", bufs=1) as consts, \
             tc.tile_pool(name="xin", bufs=2) as xin, \
             tc.tile_pool(name="sq", bufs=2) as sqp, \
             tc.tile_pool(name="work", bufs=2) as work, \
             tc.tile_pool(name="psp", bufs=2, space="PSUM") as psp, \
             tc.tile_pool(name="psl", bufs=2, space="PSUM") as psl, \
             tc.tile_pool(name="pst", bufs=2, space="PSUM") as pst:

            mpk_sb = consts.tile([P, EC, 2 * O], dt.bfloat16)
            nc.sync.dma_start(mpk_sb, mpk_d)
            n3_sb = consts.tile([3, O], dt.bfloat16)
            nc.sync.dma_start(n3_sb, n3_d)

            # first tile's x (in halves) ahead of the weights: lin
            # matmuls warm the PE while wh streams in
            xh0_sb = xin.tile([P, EC, TT], dt.bfloat16, tag="xh")
            xl0_sb = xin.tile([P, EC, TT], dt.bfloat16, tag="xl")
            nc.sync.dma_start(xh0_sb[:, :EH], xh_d[0, :, :EH])
            nc.sync.dma_start(xl0_sb[:, :EH], xl_d[0, :, :EH])
            nc.sync.dma_start(xh0_sb[:, EH:], xh_d[0, :, EH:])
            nc.sync.dma_start(xl0_sb[:, EH:], xl_d[0, :, EH:])
            wh_sb = consts.tile([P, EC, H], dt.bfloat16)
            nc.sync.dma_start(wh_sb, wh_d)

            from concourse.masks import make_identity
            ident_sb = consts.tile([P, P], dt.float32)
            make_identity(nc, ident_sb)
            ones_sb = consts.tile([P, O], dt.bfloat16)
            nc.vector.memset(ones_sb, 1.0)
            ones3_sb = consts.tile([3, TT], dt.bfloat16)
            nc.vector.memset(ones3_sb, 1.0)

            ot_sb = consts.tile([P, NTT, CC, O], dt.float32)

            for tt in range(NTT):
                if tt == 0:
                    xh_sb, xl_sb = xh0_sb, xl0_sb
                else:
                    xh_sb = xin.tile([P, EC, TT], dt.bfloat16, tag="xh")
                    xl_sb = xin.tile([P, EC, TT], dt.bfloat16, tag="xl")
                    nc.sync.dma_start(xh_sb[:, :EH], xh_d[tt, :, :EH])
                    nc.sync.dma_start(xl_sb[:, :EH], xl_d[tt, :, :EH])
                    nc.sync.dma_start(xh_sb[:, EH:], xh_d[tt, :, EH:])
                    nc.sync.dma_start(xl_sb[:, EH:], xl_d[tt, :, EH:])

                # ---- d2 accumulator in [o, t]: lin + q + n ----
                ps_l = psl.tile([O, TT], dt.float32, tag="lin")
                k = 0
                for e in range(EC):
                    for (msl, xq) in ((slice(0, O), xh_sb),
                                      (slice(O, 2 * O), xh_sb),
                                      (slice(0, O), xl_sb)):
                        nc.tensor.matmul(
                            ps_l, mpk_sb[:, e, msl], xq[:, e],
                            start=(k == 0), stop=False)
                        k += 1

                # ---- proj (W x)^T per h-chunk; fold its square into
                # ps_l one h-stage behind ----
                p2h_t = [None] * HC
                p2l_t = [None] * HC

                def emit_qfold(h):
                    nc.tensor.matmul(ps_l, ones_sb, p2h_t[h],
                                     start=False, stop=False)
                    nc.tensor.matmul(ps_l, ones_sb, p2l_t[h],
                                     start=False, stop=False)

                for h in range(HC):
                    ps_p = psp.tile([P, TT], dt.float32, tag="pp")
                    hsl = slice(h * P, (h + 1) * P)
                    k = 0
                    for e in range(EC):
                        for xq in (xh_sb, xl_sb):
                            nc.tensor.matmul(
                                ps_p, wh_sb[:, e, hsl], xq[:, e],
                                start=(k == 0), stop=(k == 2 * EC - 1))
                            k += 1
                    sq_sb = sqp.tile([P, TT], dt.float32, tag="sq")
                    nc.scalar.activation(sq_sb, ps_p,
                                         mybir.ActivationFunctionType.Square)
                    p2h_sb = sqp.tile([P, TT], dt.bfloat16, tag="p2h")
                    nc.gpsimd.tensor_scalar(p2h_sb, sq_sb, 1.0, None,
                                            mybir.AluOpType.mult)
                    p2l_sb = sqp.tile([P, TT], dt.bfloat16, tag="p2l")
                    nc.vector.tensor_tensor(p2l_sb, sq_sb, p2h_sb,
                                            mybir.AluOpType.subtract)
                    p2h_t[h], p2l_t[h] = p2h_sb, p2l_sb
                    if h >= 1:
                        emit_qfold(h - 1)
                emit_qfold(HC - 1)
                nc.tensor.matmul(ps_l, n3_sb, ones3_sb,
                                 start=False, stop=True)

                # ---- d2T: [o, t] -> [t, o] (needs SBUF staging) ----
                lin_sb = work.tile([O, TT], dt.float32, tag="lin")
                nc.scalar.copy(lin_sb, ps_l)
                ps_t = pst.tile([P, CC, O], dt.float32, tag="d2T")
                for c in range(CC):
                    nc.tensor.matmul(
                        ps_t[:, c], lin_sb[:, c * P:(c + 1) * P],
                        ident_sb[:O, :O], is_transpose=True,
                        start=True, stop=True)

                # ---- per-c tail: newton-sqrt, softmax, threshold ----
                s0_sb = work.tile([P, CC, O], dt.float32, tag="s0")
                r_sb = work.tile([P, CC, O], dt.float32, tag="r")
                u_sb = work.tile([P, CC, O], dt.float32, tag="u")
                e_sb = work.tile([P, CC, O], dt.float32, tag="e")
                ssum_sb = work.tile([P, CC], dt.float32, tag="ssum")
                rec_sb = work.tile([P, CC], dt.float32, tag="rec")
                p1_sb = work.tile([P, CC, O], dt.float32, tag="p1")
                msk_sb = work.tile([P, CC, O], dt.float32, tag="msk")

                for c in range(CC):
                    nc.scalar.activation(s0_sb[:, c], ps_t[:, c],
                                         mybir.ActivationFunctionType.Ln)
                    nc.scalar.activation(s0_sb[:, c], s0_sb[:, c],
                                         mybir.ActivationFunctionType.Exp,
                                         scale=0.5)
                    # newton: u = s0 + d2/s0; e = exp(-u/2)
                    nc.vector.reciprocal(r_sb[:, c], s0_sb[:, c])
                    nc.vector.tensor_tensor(u_sb[:, c], ps_t[:, c],
                                            r_sb[:, c],
                                            mybir.AluOpType.mult)
                    nc.gpsimd.tensor_tensor(u_sb[:, c], u_sb[:, c],
                                            s0_sb[:, c],
                                            mybir.AluOpType.add)
                    nc.scalar.activation(e_sb[:, c], u_sb[:, c],
                                         mybir.ActivationFunctionType.Exp,
                                         scale=-0.5)
                    nc.vector.reduce_sum(ssum_sb[:, c:c + 1], e_sb[:, c],
                                         axis=mybir.AxisListType.X)
                    nc.vector.reciprocal(rec_sb[:, c:c + 1],
                                         ssum_sb[:, c:c + 1])
                    nc.vector.tensor_tensor(
                        p1_sb[:, c], e_sb[:, c],
                        rec_sb[:, c:c + 1].to_broadcast((P, O)),
                        mybir.AluOpType.mult)
                    nc.gpsimd.tensor_scalar(
                        msk_sb[:, c], p1_sb[:, c], THRESH, None,
                        mybir.AluOpType.is_ge)
                    nc.gpsimd.tensor_tensor(ot_sb[:, tt, c], p1_sb[:, c],
                                            msk_sb[:, c],
                                            mybir.AluOpType.mult)

            nc.sync.dma_start(out_d, ot_sb)

    import concourse.bacc as _bacc_mod
    _orig_gat = _bacc_mod.get_activation_tables
    _KEEP = "natural_log_exp_and_others"

    def _patched_gat(arch):
        return {k: (v if k == _KEEP else set())
                for k, v in _orig_gat(arch).items()}

    _bacc_mod.get_activation_tables = _patched_gat
    try:
        nc.compile()
    finally:
        _bacc_mod.get_activation_tables = _orig_gat
    return nc


def _prep_inputs(word_output, W, b, op_emb):
    x = np.asarray(word_output, np.float32).reshape(TOK, E)
    xh, xl = _split(x)
    xth = xh.T.reshape(EC, P, NCORES, NTT, TT)
    xtl = xl.T.reshape(EC, P, NCORES, NTT, TT)

    Wt64 = np.asarray(W, np.float64).T            # [E, H]
    wh = Wt64.astype(np.float32).astype(BF)       # [E, H] bf16
    wh_t = np.ascontiguousarray(wh.reshape(EC, P, H).transpose(1, 0, 2))

    bd = np.asarray(b, np.float64)
    oed = np.asarray(op_emb, np.float64)
    m = 2.0 * ((bd[None, :] - oed) @ np.asarray(W, np.float64)).T  # [E, O]
    mh, ml = _split(m.astype(np.float32))
    mpk = np.concatenate([mh, ml], axis=1)        # [E, 64]
    mpk_t = np.ascontiguousarray(
        mpk.reshape(EC, P, 2 * O).transpose(1, 0, 2))

    n = ((bd[None, :] - oed) ** 2).sum(-1)        # [O] fp64
    na = n.astype(np.float32)
    nh = na.astype(BF)
    nm = (na - nh.astype(np.float32)).astype(BF)
    nl = (na - nh.astype(np.float32) - nm.astype(np.float32)).astype(BF)
    n3 = np.stack([nh, nm, nl], axis=0)           # [3, O] bf16

    common = {"wh": wh_t, "mpk": mpk_t, "n3": n3}
    in_maps = []
    for core in range(NCORES):
        mcore = dict(common)
        mcore["xh"] = np.ascontiguousarray(
            xth[:, :, core].transpose(2, 1, 0, 3))
        mcore["xl"] = np.ascontiguousarray(
            xtl[:, :, core].transpose(2, 1, 0, 3))
        in_maps.append(mcore)
    return in_maps


def kernel(word_output, W, b, op_emb, _trace=False):
    if "nc" not in _CACHE:
        _CACHE["nc"] = _build()
    nc = _CACHE["nc"]
    in_maps = _prep_inputs(word_output, W, b, op_emb)
    try:
        res = bass_utils.run_bass_kernel_spmd(
            nc, in_maps, core_ids=list(range(NCORES)), trace=_trace)
    except ModuleNotFoundError:
        res = bass_utils.run_bass_kernel_spmd(
            nc, in_maps, core_ids=list(range(NCORES)), trace=False)
    outs = []
    for r in res.results:
        o = r["out"]                       # [P, NTT, CC, O]
        outs.append(o.transpose(1, 2, 0, 3).reshape(TPC, O))
    out = np.concatenate(outs, axis=0)
    _CACHE["last_results"] = res
    return out.reshape(B, S, O)


if __name__ == "__main__":
    rng = np.random.default_rng(0)
    wo = rng.standard_normal((B, S, E)).astype(np.float32)
    W_ = (rng.standard_normal((H, E)) / np.sqrt(E)).astype(np.float32)
    b_ = (rng.standard_normal(H) * 0.01).astype(np.float32)
    oe = rng.standard_normal((O, H)).astype(np.float32)
    out = kernel(wo, W_, b_, oe)
    proj = wo.reshape(-1, E).astype(np.float64) @ W_.T.astype(np.float64) + b_
    diff = proj[:, None, :] - oe
    d2 = (diff * diff).sum(-1)
    dis = -np.sqrt(d2)
    ex = np.exp(dis - dis.max(-1, keepdims=True))
    prob = ex / ex.sum(-1, keepdims=True)
    ref = np.where(prob < THRESH, 0, prob).astype(np.float32).reshape(B, S, O)
    num = np.linalg.norm(out - ref)
    den = np.linalg.norm(ref)
    print("norm rel err:", num / den)
    print("max abs err:", np.abs(out - ref).max())


# revision 17
# speedup vs baseline: 1.5725x; 1.5725x over previous
"""TRN2 Bass kernel for nn_Knowledge_Base (retrieval_knn).

reference:
    proj = word_output @ W.T + b            # [B,S,H]
    dis  = -sqrt(sum((proj[...,None,:] - op_emb)**2, -1))   # [B,S,O]
    prob = softmax(dis, -1); prob[prob < 0.3] = 0

Strategy (8 cores, data-parallel over the 8192 tokens, 1024/core):
  d2(o) = ||W x||^2 + m_o . x + n_o   with host-precomputed (fp64)
      m_o = 2 W^T (b - e_o)  in R^768,   n_o = ||b - e_o||^2.
  This decouples per-o accuracy (exact m/n + 3-term bf16-split x.m
  matmul) from ||Wx||^2, whose error is shared across all 32 ops and
  attenuated ~60x by the sqrt curvature, so a 2-pass bf16 proj
  (Wh@xh + Wh@xl) suffices; the residual is an algebraic omission
  (missing Wl@x), identical on HW and in simulation (~5e-6 prob error
  at the tightest threshold margin).
  The whole d2 accumulates in ONE [o, t] PSUM tile: 18 lin passes,
  then per h-chunk the squared proj (ACT Square -> bf16 hi/lo planes)
  enters via ones-stationary matmuls (broadcasts ||Wx||^2 over o), and
  n_o enters as one K=3 matmul of its 3-way bf16 split against ones.
  Transpose d2 to [t, o] on the PE, then a 3-engine tail per 128-token
  chunk: ACT ln/exp for s0 ~ sqrt(d2), one DVE Newton step
  (bit-exact reciprocal) to kill the ~1e-4 act-table error, ACT
  exp(-u/2), DVE normalize, GPSIMD threshold.  All ACT funcs live in
  the one covering act table set (no mid-kernel table reloads).
  Host pre/post: x pre-split bf16 hi/lo, pre-tiled so every DMA is
  contiguous per partition (x tile DMAs split in halves so the PE
  starts after the first half); output written in on-chip layout and
  unpermuted on host.
"""
import sys
sys.path.insert(0, "/opt/trn_rl_repo")
import numpy as np
import ml_dtypes

import concourse.bacc as bacc
import concourse.tile as tile
from concourse import mybir
from concourse import bass_utils

BF = ml_dtypes.bfloat16
P = 128
B, S, E, H, O = 4, 2048, 768, 512, 32
NCORES = 8
TOK = B * S                  # 8192
TPC = TOK // NCORES          # 1024 tokens per core
TT = 512                     # t-tile size
NTT = TPC // TT              # 2 t-tiles per core
EC = E // P                  # 6 e-chunks
HC = H // P                  # 4 h-chunks
CC = TT // P                 # 4 c-chunks (token blocks) per t-tile
EH = 3                       # e-chunks in first DMA half
THRESH = 0.3

_CACHE = {}


def _split(a):
    hi = a.astype(BF)
    lo = (a.astype(np.float32) - hi.astype(np.float32)).astype(BF)
    return hi, lo


def _build():
    nc = bacc.Bacc("TRN2", target_bir_lowering=False, debug=False,
                   num_devices=NCORES)
    dt = mybir.dt
    xh_d = nc.dram_tensor("xh", [NTT, P, EC, TT], dt.bfloat16,
                          kind="ExternalInput").ap()
    xl_d = nc.dram_tensor("xl", [NTT, P, EC, TT], dt.bfloat16,
                          kind="ExternalInput").ap()
    wh_d = nc.dram_tensor("wh", [P, EC, H], dt.bfloat16,
                          kind="ExternalInput").ap()
    mpk_d = nc.dram_tensor("mpk", [P, EC, 2 * O], dt.bfloat16,
                           kind="ExternalInput").ap()
    # n3: [3, O] bf16 three-way split of ||b - e_o||^2
    n3_d = nc.dram_tensor("n3", [3, O], dt.bfloat16,
                          kind="ExternalInput").ap()
    out_d = nc.dram_tensor("out", [P, NTT, CC, O], dt.float32,
                           kind="ExternalOutput").ap()

    with tile.TileContext(nc) as tc:
        with tc.tile_pool(name="consts", bufs=1) as consts, \
             tc.tile_pool(name="xin", bufs=2) as xin, \
             tc.tile_pool(name="sq", bufs=2) as sqp, \
             tc.tile_pool(name="work", bufs=2) as work, \
             tc.tile_pool(name="psp", bufs=2, space="PSUM") as psp, \
             tc.tile_pool(name="psl", bufs=2, space="PSUM") as psl, \
             tc.tile_pool(name="pst", bufs=2, space="PSUM") as pst:

            mpk_sb = consts.tile([P, EC, 2 * O], dt.bfloat16)
            nc.sync.dma_start(mpk_sb, mpk_d)
            n3_sb = consts.tile([3, O], dt.bfloat16)
            nc.sync.dma_start(n3_sb, n3_d)

            # first tile's x (in halves) ahead of the weights: lin
            # matmuls warm the PE while wh streams in
            xh0_sb = xin.tile([P, EC, TT], dt.bfloat16, tag="xh")
            xl0_sb = xin.tile([P, EC, TT], dt.bfloat16, tag="xl")
            nc.sync.dma_start(xh0_sb[:, :EH], xh_d[0, :, :EH])
            nc.sync.dma_start(xl0_sb[:, :EH], xl_d[0, :, :EH])
            nc.sync.dma_start(xh0_sb[:, EH:], xh_d[0, :, EH:])
            nc.sync.dma_start(xl0_sb[:, EH:], xl_d[0, :, EH:])
            wh_sb = consts.tile([P, EC, H], dt.bfloat16)
            nc.sync.dma_start(wh_sb, wh_d)

            from concourse.masks import make_identity
            ident_sb = consts.tile([P, P], dt.float32)
            make_identity(nc, ident_sb)
            ones_sb = consts.tile([P, O], dt.float16)
            nc.vector.memset(ones_sb, 1.0)
            ones3_sb = consts.tile([3, TT], dt.bfloat16)
            nc.vector.memset(ones3_sb, 1.0)

            ot_sb = consts.tile([P, NTT, CC, O], dt.float32)

            for tt in range(NTT):
                if tt == 0:
                    xh_sb, xl_sb = xh0_sb, xl0_sb
                else:
                    xh_sb = xin.tile([P, EC, TT], dt.bfloat16, tag="xh")
                    xl_sb = xin.tile([P, EC, TT], dt.bfloat16, tag="xl")
                    nc.sync.dma_start(xh_sb[:, :EH], xh_d[tt, :, :EH])
                    nc.sync.dma_start(xl_sb[:, :EH], xl_d[tt, :, :EH])
                    nc.sync.dma_start(xh_sb[:, EH:], xh_d[tt, :, EH:])
                    nc.sync.dma_start(xl_sb[:, EH:], xl_d[tt, :, EH:])

                # ---- d2 accumulator in [o, t]: lin + q + n ----
                ps_l = psl.tile([O, TT], dt.float32, tag="lin")
                k = 0
                for e in range(EC):
                    for (msl, xq) in ((slice(0, O), xh_sb),
                                      (slice(O, 2 * O), xh_sb),
                                      (slice(0, O), xl_sb)):
                        nc.tensor.matmul(
                            ps_l, mpk_sb[:, e, msl], xq[:, e],
                            start=(k == 0), stop=False)
                        k += 1

                # ---- proj (W x)^T per h-chunk; fold its square into
                # ps_l one h-stage behind (single fp16 plane: 10
                # mantissa bits keep the shared-q error ~16x under the
                # flip threshold; ACT converts at line rate) ----
                p2_t = [None] * HC

                def emit_qfold(h):
                    nc.tensor.matmul(ps_l, ones_sb, p2_t[h],
                                     start=False, stop=False)

                for h in range(HC):
                    ps_p = psp.tile([P, TT], dt.float32, tag="pp")
                    hsl = slice(h * P, (h + 1) * P)
                    k = 0
                    for e in range(EC):
                        for xq in (xh_sb, xl_sb):
                            nc.tensor.matmul(
                                ps_p, wh_sb[:, e, hsl], xq[:, e],
                                start=(k == 0), stop=(k == 2 * EC - 1))
                            k += 1
                    p2_sb = sqp.tile([P, TT], dt.float16, tag="p2")
                    nc.scalar.activation(p2_sb, ps_p,
                                         mybir.ActivationFunctionType.Square)
                    p2_t[h] = p2_sb
                    if h >= 1:
                        emit_qfold(h - 1)
                emit_qfold(HC - 1)
                nc.tensor.matmul(ps_l, n3_sb, ones3_sb,
                                 start=False, stop=True)

                # ---- d2T: [o, t] -> [t, o] (needs SBUF staging) ----
                lin_sb = work.tile([O, TT], dt.float32, tag="lin")
                nc.scalar.copy(lin_sb, ps_l)
                ps_t = pst.tile([P, CC, O], dt.float32, tag="d2T")
                for c in range(CC):
                    nc.tensor.matmul(
                        ps_t[:, c], lin_sb[:, c * P:(c + 1) * P],
                        ident_sb[:O, :O], is_transpose=True,
                        start=True, stop=True)

                # ---- per-c tail: newton-sqrt, softmax, threshold ----
                s0_sb = work.tile([P, CC, O], dt.float32, tag="s0")
                r_sb = work.tile([P, CC, O], dt.float32, tag="r")
                u_sb = work.tile([P, CC, O], dt.float32, tag="u")
                e_sb = work.tile([P, CC, O], dt.float32, tag="e")
                ssum_sb = work.tile([P, CC], dt.float32, tag="ssum")
                rec_sb = work.tile([P, CC], dt.float32, tag="rec")
                p1_sb = work.tile([P, CC, O], dt.float32, tag="p1")
                msk_sb = work.tile([P, CC, O], dt.float32, tag="msk")

                for c in range(CC):
                    nc.scalar.activation(s0_sb[:, c], ps_t[:, c],
                                         mybir.ActivationFunctionType.Ln)
                    nc.scalar.activation(s0_sb[:, c], s0_sb[:, c],
                                         mybir.ActivationFunctionType.Exp,
                                         scale=0.5)
                    # newton: u = s0 + d2/s0; e = exp(-u/2)
                    nc.vector.reciprocal(r_sb[:, c], s0_sb[:, c])
                    nc.vector.tensor_tensor(u_sb[:, c], ps_t[:, c],
                                            r_sb[:, c],
                                            mybir.AluOpType.mult)
                    nc.gpsimd.tensor_tensor(u_sb[:, c], u_sb[:, c],
                                            s0_sb[:, c],
                                            mybir.AluOpType.add)
                    # (gpsimd is fine at [128,32]; never give it big tiles)
                    nc.scalar.activation(e_sb[:, c], u_sb[:, c],
                                         mybir.ActivationFunctionType.Exp,
                                         scale=-0.5)
                    nc.vector.reduce_sum(ssum_sb[:, c:c + 1], e_sb[:, c],
                                         axis=mybir.AxisListType.X)
                    nc.vector.reciprocal(rec_sb[:, c:c + 1],
                                         ssum_sb[:, c:c + 1])
                    nc.vector.tensor_tensor(
                        p1_sb[:, c], e_sb[:, c],
                        rec_sb[:, c:c + 1].to_broadcast((P, O)),
                        mybir.AluOpType.mult)
                    nc.gpsimd.tensor_scalar(
                        msk_sb[:, c], p1_sb[:, c], THRESH, None,
                        mybir.AluOpType.is_ge)
                    nc.gpsimd.tensor_tensor(ot_sb[:, tt, c], p1_sb[:, c],
                                            msk_sb[:, c],
                                            mybir.AluOpType.mult)

            nc.sync.dma_start(out_d, ot_sb)

    import concourse.bacc as _bacc_mod
    _orig_gat = _bacc_mod.get_activation_tables
    _KEEP = "natural_log_exp_and_others"

    def _patched_gat(arch):
        return {k: (v if k == _KEEP else set())
                for k, v in _orig_gat(arch).items()}

    _bacc_mod.get_activation_tables = _patched_gat
    try:
        nc.compile()
    finally:
        _bacc_mod.get_activation_tables = _orig_gat
    return nc


def _prep_inputs(word_output, W, b, op_emb):
    x = np.asarray(word_output, np.float32).reshape(TOK, E)
    xh, xl = _split(x)
    xth = xh.T.reshape(EC, P, NCORES, NTT, TT)
    xtl = xl.T.reshape(EC, P, NCORES, NTT, TT)

    Wt64 = np.asarray(W, np.float64).T            # [E, H]
    wh = Wt64.astype(np.float32).astype(BF)       # [E, H] bf16
    wh_t = np.ascontiguousarray(wh.reshape(EC, P, H).transpose(1, 0, 2))

    bd = np.asarray(b, np.float64)
    oed = np.asarray(op_emb, np.float64)
    m = 2.0 * ((bd[None, :] - oed) @ np.asarray(W, np.float64)).T  # [E, O]
    mh, ml = _split(m.astype(np.float32))
    mpk = np.concatenate([mh, ml], axis=1)        # [E, 64]
    mpk_t = np.ascontiguousarray(
        mpk.reshape(EC, P, 2 * O).transpose(1, 0, 2))

    n = ((bd[None, :] - oed) ** 2).sum(-1)        # [O] fp64
    na = n.astype(np.float32)
    nh = na.astype(BF)
    nm = (na - nh.astype(np.float32)).astype(BF)
    nl = (na - nh.astype(np.float32) - nm.astype(np.float32)).astype(BF)
    n3 = np.stack([nh, nm, nl], axis=0)           # [3, O] bf16

    common = {"wh": wh_t, "mpk": mpk_t, "n3": n3}
    in_maps = []
    for core in range(NCORES):
        mcore = dict(common)
        mcore["xh"] = np.ascontiguousarray(
            xth[:, :, core].transpose(2, 1, 0, 3))
        mcore["xl"] = np.ascontiguousarray(
            xtl[:, :, core].transpose(2, 1, 0, 3))
        in_maps.append(mcore)
    return in_maps


def kernel(word_output, W, b, op_emb, _trace=False):
    if "nc" not in _CACHE:
        _CACHE["nc"] = _build()
    nc = _CACHE["nc"]
    in_maps = _prep_inputs(word_output, W, b, op_emb)
    try:
        res = bass_utils.run_bass_kernel_spmd(
            nc, in_maps, core_ids=list(range(NCORES)), trace=_trace)
    except ModuleNotFoundError:
        res = bass_utils.run_bass_kernel_spmd(
            nc, in_maps, core_ids=list(range(NCORES)), trace=False)
    outs = []
    for r in res.results:
        o = r["out"]                       # [P, NTT, CC, O]
        outs.append(o.transpose(1, 2, 0, 3).reshape(TPC, O))
    out = np.concatenate(outs, axis=0)
    _CACHE["last_results"] = res
    return out.reshape(B, S, O)


if __name__ == "__main__":
    rng = np.random.default_rng(0)
    wo = rng.standard_normal((B, S, E)).astype(np.float32)
    W_ = (rng.standard_normal((H, E)) / np.sqrt(E)).astype(np.float32)
    b_ = (rng.standard_normal(H) * 0.01).astype(np.float32)
    oe = rng.standard_normal((O, H)).astype(np.float32)
    out = kernel(wo, W_, b_, oe)
    proj = wo.reshape(-1, E).astype(np.float64) @ W_.T.astype(np.float64) + b_
    diff = proj[:, None, :] - oe
    d2 = (diff * diff).sum(-1)
    dis = -np.sqrt(d2)
    ex = np.exp(dis - dis.max(-1, keepdims=True))
    prob = ex / ex.sum(-1, keepdims=True)
    ref = np.where(prob < THRESH, 0, prob).astype(np.float32).reshape(B, S, O)
    num = np.linalg.norm(out - ref)
    den = np.linalg.norm(ref)
    print("norm rel err:", num / den)
    print("max abs err:", np.abs(out - ref).max())


# revision 18
# speedup vs baseline: 2.0109x; 1.2788x over previous
"""TRN2 Bass kernel for nn_Knowledge_Base (retrieval_knn).

reference:
    proj = word_output @ W.T + b            # [B,S,H]
    dis  = -sqrt(sum((proj[...,None,:] - op_emb)**2, -1))   # [B,S,O]
    prob = softmax(dis, -1); prob[prob < 0.3] = 0

Strategy (8 cores, data-parallel over the 8192 tokens, 1024/core):
  d2(o) = ||W x||^2 + m_o . x + n_o   with host-precomputed (fp64)
      m_o = 2 W^T (b - e_o)  in R^768,   n_o = ||b - e_o||^2.
  This decouples per-o accuracy (exact m/n + 3-term bf16-split x.m
  matmul) from ||Wx||^2, whose error is shared across all 32 ops and
  attenuated ~60x by the sqrt curvature, so a 2-pass bf16 proj
  (Wh@xh + Wh@xl) suffices; the residual is an algebraic omission
  (missing Wl@x), identical on HW and in simulation (~5e-6 prob error
  at the tightest threshold margin, 0 threshold flips).
  ||Wx||^2: transpose proj via PE identity matmuls into [t, h] PSUM,
  ACT Square + accum row-sum.  4 t-tiles of 256 tokens pipeline the
  per-tile softmax tail under the next tile's matmuls, so only the
  last 256-token tail is exposed.
  sqrt: s0 = exp(0.5 ln d2) from the ACT table, then one DVE Newton
  step u = s0 + d2/s0 (bit-exact reciprocal), final ACT exp(-0.5 u).
  The Newton step removes the ~1e-4 act-table error that otherwise
  flips threshold-marginal probs.  All ACT funcs (copy, square, ln,
  exp) are forced into the one covering act table set, so there are
  no mid-kernel table reloads.
  Host pre/post: x pre-split bf16 hi/lo and pre-tiled so every DMA is
  contiguous per partition (first tile in halves so the PE starts
  early); output written in on-chip [p, c, o] layout, host unpermutes.
"""
import sys
sys.path.insert(0, "/opt/trn_rl_repo")
import numpy as np
import ml_dtypes

import concourse.bacc as bacc
import concourse.tile as tile
from concourse import mybir
from concourse import bass_utils

BF = ml_dtypes.bfloat16
P = 128
B, S, E, H, O = 4, 2048, 768, 512, 32
NCORES = 8
TOK = B * S                  # 8192
TPC = TOK // NCORES          # 1024 tokens per core
TT = 256                     # t-tile size
NTT = TPC // TT              # 4 t-tiles per core
EC = E // P                  # 6 e-chunks
HC = H // P                  # 4 h-chunks
CC = TT // P                 # 2 c-chunks (token blocks) per t-tile
EH = 3                       # e-chunks in first DMA half
THRESH = 0.3

_CACHE = {}


def _split(a):
    hi = a.astype(BF)
    lo = (a.astype(np.float32) - hi.astype(np.float32)).astype(BF)
    return hi, lo


def _build():
    nc = bacc.Bacc("TRN2", target_bir_lowering=False, debug=False,
                   num_devices=NCORES)
    dt = mybir.dt
    xh_d = nc.dram_tensor("xh", [NTT, P, EC, TT], dt.bfloat16,
                          kind="ExternalInput").ap()
    xl_d = nc.dram_tensor("xl", [NTT, P, EC, TT], dt.bfloat16,
                          kind="ExternalInput").ap()
    wh_d = nc.dram_tensor("wh", [P, EC, H], dt.bfloat16,
                          kind="ExternalInput").ap()
    mpk_d = nc.dram_tensor("mpk", [P, EC, 2 * O], dt.bfloat16,
                           kind="ExternalInput").ap()
    n128_d = nc.dram_tensor("n128", [P, O], dt.float32,
                            kind="ExternalInput").ap()
    out_d = nc.dram_tensor("out", [P, NTT, CC, O], dt.float32,
                           kind="ExternalOutput").ap()

    with tile.TileContext(nc) as tc:
        with tc.tile_pool(name="consts", bufs=1) as consts, \
             tc.tile_pool(name="xin", bufs=2) as xin, \
             tc.tile_pool(name="proj", bufs=2) as proj, \
             tc.tile_pool(name="work", bufs=2) as work, \
             tc.tile_pool(name="psp", bufs=2, space="PSUM") as psp, \
             tc.tile_pool(name="psl", bufs=2, space="PSUM") as psl, \
             tc.tile_pool(name="psn", bufs=1, space="PSUM") as psn, \
             tc.tile_pool(name="pst", bufs=2, space="PSUM") as pst:

            mpk_sb = consts.tile([P, EC, 2 * O], dt.bfloat16)
            nc.sync.dma_start(mpk_sb, mpk_d)
            n128_sb = consts.tile([P, O], dt.float32)
            nc.sync.dma_start(n128_sb, n128_d)

            # first tile's x (in halves) ahead of the weights: lin
            # matmuls warm the PE while wh streams in
            xh0_sb = xin.tile([P, EC, TT], dt.bfloat16, tag="xh")
            xl0_sb = xin.tile([P, EC, TT], dt.bfloat16, tag="xl")
            nc.sync.dma_start(xh0_sb[:, :EH], xh_d[0, :, :EH])
            nc.sync.dma_start(xl0_sb[:, :EH], xl_d[0, :, :EH])
            nc.sync.dma_start(xh0_sb[:, EH:], xh_d[0, :, EH:])
            nc.sync.dma_start(xl0_sb[:, EH:], xl_d[0, :, EH:])
            wh_sb = consts.tile([P, EC, H], dt.bfloat16)
            nc.sync.dma_start(wh_sb, wh_d)

            from concourse.masks import make_identity
            ident_sb = consts.tile([P, P], dt.float32)
            make_identity(nc, ident_sb)

            ot_sb = consts.tile([P, NTT, CC, O], dt.float32)

            for tt in range(NTT):
                if tt == 0:
                    xh_sb, xl_sb = xh0_sb, xl0_sb
                else:
                    xh_sb = xin.tile([P, EC, TT], dt.bfloat16, tag="xh")
                    xl_sb = xin.tile([P, EC, TT], dt.bfloat16, tag="xl")
                    nc.sync.dma_start(xh_sb, xh_d[tt])
                    nc.sync.dma_start(xl_sb, xl_d[tt])

                # ---- lin: m_o . x  (3-term split) in [o, t] ----
                ps_l = psl.tile([O, TT], dt.float32, tag="lin")
                k = 0
                for e in range(EC):
                    for (msl, xq) in ((slice(0, O), xh_sb),
                                      (slice(O, 2 * O), xh_sb),
                                      (slice(0, O), xl_sb)):
                        nc.tensor.matmul(
                            ps_l, mpk_sb[:, e, msl], xq[:, e],
                            start=(k == 0), stop=(k == 3 * EC - 1))
                        k += 1
                lin_sb = work.tile([O, TT], dt.float32, tag="lin")
                nc.scalar.copy(lin_sb, ps_l)

                # ---- linT: [o, t] -> [t, o] ----
                ps_t = pst.tile([P, CC, O], dt.float32, tag="linT")
                for c in range(CC):
                    nc.tensor.matmul(
                        ps_t[:, c], lin_sb[:, c * P:(c + 1) * P],
                        ident_sb[:O, :O], is_transpose=True,
                        start=True, stop=True)

                # ---- proj (W x)^T in [h, t], 2-pass, with the norm
                # transposes software-pipelined one h-stage behind ----
                pf_sb = proj.tile([P, HC, TT], dt.float32, tag="pf")
                ps_ns = [psn.tile([P, H], dt.float32, tag=f"pn{c}",
                                  name=f"ps_n{c}")
                         for c in range(CC)]

                def emit_normT(h):
                    hosl = slice(h * P, (h + 1) * P)
                    for c in range(CC):
                        nc.tensor.matmul(
                            ps_ns[c][:, hosl],
                            pf_sb[:, h, c * P:(c + 1) * P], ident_sb,
                            is_transpose=True, start=True,
                            stop=(h == HC - 1))

                for h in range(HC):
                    ps_p = psp.tile([P, TT], dt.float32, tag="pp")
                    hsl = slice(h * P, (h + 1) * P)
                    k = 0
                    for e in range(EC):
                        for xq in (xh_sb, xl_sb):
                            nc.tensor.matmul(
                                ps_p, wh_sb[:, e, hsl], xq[:, e],
                                start=(k == 0), stop=(k == 2 * EC - 1))
                            k += 1
                    nc.scalar.copy(pf_sb[:, h], ps_p)
                    if h >= 1:
                        emit_normT(h - 1)
                emit_normT(HC - 1)

                # ---- per-c tail: q, d2, newton-sqrt, softmax ----
                junk_sb = work.tile([P, H], dt.float32, tag="junk")
                normp_sb = work.tile([P, CC], dt.float32, tag="normp")
                d2_sb = work.tile([P, CC, O], dt.float32, tag="d2")
                s0_sb = work.tile([P, CC, O], dt.float32, tag="s0")
                r_sb = work.tile([P, CC, O], dt.float32, tag="r")
                u_sb = work.tile([P, CC, O], dt.float32, tag="u")
                e_sb = work.tile([P, CC, O], dt.float32, tag="e")
                ssum_sb = work.tile([P, CC], dt.float32, tag="ssum")
                rec_sb = work.tile([P, CC], dt.float32, tag="rec")
                p1_sb = work.tile([P, CC, O], dt.float32, tag="p1")
                msk_sb = work.tile([P, CC, O], dt.float32, tag="msk")

                for c in range(CC):
                    nc.scalar.activation(
                        junk_sb, ps_ns[c],
                        mybir.ActivationFunctionType.Square,
                        accum_out=normp_sb[:, c:c + 1])
                    nc.vector.tensor_tensor(
                        d2_sb[:, c], ps_t[:, c],
                        n128_sb, mybir.AluOpType.add)
                    nc.vector.tensor_tensor(
                        d2_sb[:, c], d2_sb[:, c],
                        normp_sb[:, c:c + 1].to_broadcast((P, O)),
                        mybir.AluOpType.add)
                    # s0 ~ sqrt(d2) from tables
                    nc.scalar.activation(s0_sb[:, c], d2_sb[:, c],
                                         mybir.ActivationFunctionType.Ln)
                    nc.scalar.activation(s0_sb[:, c], s0_sb[:, c],
                                         mybir.ActivationFunctionType.Exp,
                                         scale=0.5)
                    # newton: u = s0 + d2/s0; e = exp(-u/2)
                    nc.vector.reciprocal(r_sb[:, c], s0_sb[:, c])
                    nc.vector.tensor_tensor(u_sb[:, c], d2_sb[:, c],
                                            r_sb[:, c],
                                            mybir.AluOpType.mult)
                    nc.vector.tensor_tensor(u_sb[:, c], u_sb[:, c],
                                            s0_sb[:, c],
                                            mybir.AluOpType.add)
                    nc.scalar.activation(e_sb[:, c], u_sb[:, c],
                                         mybir.ActivationFunctionType.Exp,
                                         scale=-0.5)
                    nc.vector.reduce_sum(ssum_sb[:, c:c + 1], e_sb[:, c],
                                         axis=mybir.AxisListType.X)
                    nc.vector.reciprocal(rec_sb[:, c:c + 1],
                                         ssum_sb[:, c:c + 1])
                    nc.vector.tensor_tensor(
                        p1_sb[:, c], e_sb[:, c],
                        rec_sb[:, c:c + 1].to_broadcast((P, O)),
                        mybir.AluOpType.mult)
                    nc.vector.tensor_scalar(
                        msk_sb[:, c], p1_sb[:, c], THRESH, None,
                        mybir.AluOpType.is_ge)
                    nc.vector.tensor_tensor(ot_sb[:, tt, c], p1_sb[:, c],
                                            msk_sb[:, c],
                                            mybir.AluOpType.mult)

            nc.sync.dma_start(out_d, ot_sb)

    import concourse.bacc as _bacc_mod
    _orig_gat = _bacc_mod.get_activation_tables
    _KEEP = "natural_log_exp_and_others"

    def _patched_gat(arch):
        return {k: (v if k == _KEEP else set())
                for k, v in _orig_gat(arch).items()}

    _bacc_mod.get_activation_tables = _patched_gat
    try:
        nc.compile()
    finally:
        _bacc_mod.get_activation_tables = _orig_gat
    return nc


def _prep_inputs(word_output, W, b, op_emb):
    x = np.asarray(word_output, np.float32).reshape(TOK, E)
    xh, xl = _split(x)
    xth = xh.T.reshape(EC, P, NCORES, NTT, TT)
    xtl = xl.T.reshape(EC, P, NCORES, NTT, TT)

    Wt64 = np.asarray(W, np.float64).T            # [E, H]
    wh = Wt64.astype(np.float32).astype(BF)       # [E, H] bf16
    wh_t = np.ascontiguousarray(wh.reshape(EC, P, H).transpose(1, 0, 2))

    bd = np.asarray(b, np.float64)
    oed = np.asarray(op_emb, np.float64)
    m = 2.0 * ((bd[None, :] - oed) @ np.asarray(W, np.float64)).T  # [E, O]
    mh, ml = _split(m.astype(np.float32))
    mpk = np.concatenate([mh, ml], axis=1)        # [E, 64]
    mpk_t = np.ascontiguousarray(
        mpk.reshape(EC, P, 2 * O).transpose(1, 0, 2))

    n = ((bd[None, :] - oed) ** 2).sum(-1).astype(np.float32)      # [O]
    n128 = np.broadcast_to(n, (P, O)).copy()

    common = {"wh": wh_t, "mpk": mpk_t, "n128": n128}
    in_maps = []
    for core in range(NCORES):
        mcore = dict(common)
        mcore["xh"] = np.ascontiguousarray(
            xth[:, :, core].transpose(2, 1, 0, 3))
        mcore["xl"] = np.ascontiguousarray(
            xtl[:, :, core].transpose(2, 1, 0, 3))
        in_maps.append(mcore)
    return in_maps


def kernel(word_output, W, b, op_emb, _trace=False):
    if "nc" not in _CACHE:
        _CACHE["nc"] = _build()
    nc = _CACHE["nc"]
    in_maps = _prep_inputs(word_output, W, b, op_emb)
    try:
        res = bass_utils.run_bass_kernel_spmd(
            nc, in_maps, core_ids=list(range(NCORES)), trace=_trace)
    except ModuleNotFoundError:
        res = bass_utils.run_bass_kernel_spmd(
            nc, in_maps, core_ids=list(range(NCORES)), trace=False)
    outs = []
    for r in res.results:
        o = r["out"]                       # [P, NTT, CC, O]
        outs.append(o.transpose(1, 2, 0, 3).reshape(TPC, O))
    out = np.concatenate(outs, axis=0)
    _CACHE["last_results"] = res
    return out.reshape(B, S, O)


if __name__ == "__main__":
    rng = np.random.default_rng(0)
    wo = rng.standard_normal((B, S, E)).astype(np.float32)
    W_ = (rng.standard_normal((H, E)) / np.sqrt(E)).astype(np.float32)
    b_ = (rng.standard_normal(H) * 0.01).astype(np.float32)
    oe = rng.standard_normal((O, H)).astype(np.float32)
    out = kernel(wo, W_, b_, oe)
    proj = wo.reshape(-1, E).astype(np.float64) @ W_.T.astype(np.float64) + b_
    diff = proj[:, None, :] - oe
    d2 = (diff * diff).sum(-1)
    dis = -np.sqrt(d2)
    ex = np.exp(dis - dis.max(-1, keepdims=True))
    prob = ex / ex.sum(-1, keepdims=True)
    ref = np.where(prob < THRESH, 0, prob).astype(np.float32).reshape(B, S, O)
    num = np.linalg.norm(out - ref)
    den = np.linalg.norm(ref)
    print("norm rel err:", num / den)
    print("max abs err:", np.abs(out - ref).max())


# revision 21
# speedup vs baseline: 2.0257x; 1.0073x over previous
"""TRN2 Bass kernel for nn_Knowledge_Base (retrieval_knn).

reference:
    proj = word_output @ W.T + b            # [B,S,H]
    dis  = -sqrt(sum((proj[...,None,:] - op_emb)**2, -1))   # [B,S,O]
    prob = softmax(dis, -1); prob[prob < 0.3] = 0

Strategy (8 cores, data-parallel over the 8192 tokens, 1024/core):
  d2(o) = ||W x||^2 + m_o . x + n_o   with host-precomputed (fp64)
      m_o = 2 W^T (b - e_o)  in R^768,   n_o = ||b - e_o||^2.
  This decouples per-o accuracy (exact m/n + 3-term bf16-split x.m
  matmul) from ||Wx||^2, whose error is shared across all 32 ops and
  attenuated ~60x by the sqrt curvature, so a 2-pass bf16 proj
  (Wh@xh + Wh@xl) suffices; the residual is an algebraic omission
  (missing Wl@x), identical on HW and in simulation (~5e-6 prob error
  at the tightest threshold margin, 0 threshold flips).
  ||Wx||^2: transpose proj via PE identity matmuls into [t, h] PSUM,
  ACT Square + accum row-sum.  4 t-tiles of 256 tokens pipeline the
  per-tile softmax tail under the next tile's matmuls, so only the
  last 256-token tail is exposed.
  sqrt: s0 = exp(0.5 ln d2) from the ACT table, then one DVE Newton
  step u = s0 + d2/s0 (bit-exact reciprocal), final ACT exp(-0.5 u).
  The Newton step removes the ~1e-4 act-table error that otherwise
  flips threshold-marginal probs.  All ACT funcs (copy, square, ln,
  exp) are forced into the one covering act table set, so there are
  no mid-kernel table reloads.
  Host pre/post: x pre-split bf16 hi/lo and pre-tiled so every DMA is
  contiguous per partition (first tile in halves so the PE starts
  early); output written in on-chip [p, c, o] layout, host unpermutes.
"""
import sys
sys.path.insert(0, "/opt/trn_rl_repo")
import numpy as np
import ml_dtypes

import concourse.bacc as bacc
import concourse.tile as tile
from concourse import mybir
from concourse import bass_utils

BF = ml_dtypes.bfloat16
P = 128
B, S, E, H, O = 4, 2048, 768, 512, 32
NCORES = 8
TOK = B * S                  # 8192
TPC = TOK // NCORES          # 1024 tokens per core
TT = 256                     # t-tile size
NTT = TPC // TT              # 4 t-tiles per core
EC = E // P                  # 6 e-chunks
HC = H // P                  # 4 h-chunks
CC = TT // P                 # 2 c-chunks (token blocks) per t-tile
EH = 2                       # e-chunks in first DMA half
THRESH = 0.3

_CACHE = {}


def _split(a):
    hi = a.astype(BF)
    lo = (a.astype(np.float32) - hi.astype(np.float32)).astype(BF)
    return hi, lo


def _build():
    nc = bacc.Bacc("TRN2", target_bir_lowering=False, debug=False,
                   num_devices=NCORES)
    dt = mybir.dt
    xh_d = nc.dram_tensor("xh", [NTT, P, EC, TT], dt.bfloat16,
                          kind="ExternalInput").ap()
    xl_d = nc.dram_tensor("xl", [NTT, P, EC, TT], dt.bfloat16,
                          kind="ExternalInput").ap()
    wh_d = nc.dram_tensor("wh", [P, EC, H], dt.bfloat16,
                          kind="ExternalInput").ap()
    mpk_d = nc.dram_tensor("mpk", [P, EC, 2 * O], dt.bfloat16,
                           kind="ExternalInput").ap()
    n128_d = nc.dram_tensor("n128", [P, O], dt.float32,
                            kind="ExternalInput").ap()
    out_d = nc.dram_tensor("out", [P, NTT, CC, O], dt.float32,
                           kind="ExternalOutput").ap()

    with tile.TileContext(nc) as tc:
        with tc.tile_pool(name="consts", bufs=1) as consts, \
             tc.tile_pool(name="xin", bufs=2) as xin, \
             tc.tile_pool(name="proj", bufs=2) as proj, \
             tc.tile_pool(name="work", bufs=2) as work, \
             tc.tile_pool(name="psp", bufs=2, space="PSUM") as psp, \
             tc.tile_pool(name="psl", bufs=2, space="PSUM") as psl, \
             tc.tile_pool(name="psn", bufs=1, space="PSUM") as psn, \
             tc.tile_pool(name="pst", bufs=2, space="PSUM") as pst:

            mpk_sb = consts.tile([P, EC, 2 * O], dt.bfloat16)
            nc.sync.dma_start(mpk_sb, mpk_d)
            n128_sb = consts.tile([P, O], dt.float32)
            nc.sync.dma_start(n128_sb, n128_d)

            # first tile's x (in halves) ahead of the weights: lin
            # matmuls warm the PE while wh streams in
            xh0_sb = xin.tile([P, EC, TT], dt.bfloat16, tag="xh")
            xl0_sb = xin.tile([P, EC, TT], dt.bfloat16, tag="xl")
            nc.sync.dma_start(xh0_sb[:, :EH], xh_d[0, :, :EH])
            nc.sync.dma_start(xl0_sb[:, :EH], xl_d[0, :, :EH])
            nc.sync.dma_start(xh0_sb[:, EH:], xh_d[0, :, EH:])
            nc.sync.dma_start(xl0_sb[:, EH:], xl_d[0, :, EH:])
            wh_sb = consts.tile([P, EC, H], dt.bfloat16)
            nc.sync.dma_start(wh_sb, wh_d)

            from concourse.masks import make_identity
            ident_sb = consts.tile([P, P], dt.float32)
            make_identity(nc, ident_sb)

            ot_sb = consts.tile([P, NTT, CC, O], dt.float32)

            for tt in range(NTT):
                if tt == 0:
                    xh_sb, xl_sb = xh0_sb, xl0_sb
                else:
                    xh_sb = xin.tile([P, EC, TT], dt.bfloat16, tag="xh")
                    xl_sb = xin.tile([P, EC, TT], dt.bfloat16, tag="xl")
                    nc.sync.dma_start(xh_sb, xh_d[tt])
                    nc.sync.dma_start(xl_sb, xl_d[tt])

                # ---- lin: m_o . x  (3-term split) in [o, t]; the
                # [mh|ml] pair rides one M=64 stationary so xh streams
                # once; rows O:2O hold the xh.ml half, merged after the
                # transpose.  xl.mh passes target rows 0:O only; the
                # last M=64 pass carries the group stop. ----
                ps_l = psl.tile([2 * O, TT], dt.float32, tag="lin")
                nc.tensor.matmul(ps_l, mpk_sb[:, 0, :], xh_sb[:, 0],
                                 start=True, stop=False)
                for e in range(EC):
                    nc.tensor.matmul(
                        ps_l[:O], mpk_sb[:, e, :O], xl_sb[:, e],
                        start=False, stop=False, skip_group_check=True)
                for e in range(1, EC):
                    nc.tensor.matmul(ps_l, mpk_sb[:, e, :], xh_sb[:, e],
                                     start=False, stop=(e == EC - 1),
                                     skip_group_check=True)
                lin_sb = work.tile([2 * O, TT], dt.float32, tag="lin")
                nc.scalar.copy(lin_sb, ps_l)

                # ---- linT: [o, t] -> [t, o] ----
                ps_t = pst.tile([P, CC, 2 * O], dt.float32, tag="linT")
                for c in range(CC):
                    nc.tensor.matmul(
                        ps_t[:, c], lin_sb[:, c * P:(c + 1) * P],
                        ident_sb[:2 * O, :2 * O], is_transpose=True,
                        start=True, stop=True)

                # ---- proj (W x)^T in [h, t], 2-pass, with the norm
                # transposes software-pipelined one h-stage behind ----
                pf_sb = proj.tile([P, HC, TT], dt.float32, tag="pf")
                ps_ns = [psn.tile([P, H], dt.float32, tag=f"pn{c}",
                                  name=f"ps_n{c}")
                         for c in range(CC)]

                def emit_normT(h):
                    hosl = slice(h * P, (h + 1) * P)
                    for c in range(CC):
                        nc.tensor.matmul(
                            ps_ns[c][:, hosl],
                            pf_sb[:, h, c * P:(c + 1) * P], ident_sb,
                            is_transpose=True, start=True,
                            stop=(h == HC - 1))

                for h in range(HC):
                    ps_p = psp.tile([P, TT], dt.float32, tag="pp")
                    hsl = slice(h * P, (h + 1) * P)
                    k = 0
                    for e in range(EC):
                        for xq in (xh_sb, xl_sb):
                            nc.tensor.matmul(
                                ps_p, wh_sb[:, e, hsl], xq[:, e],
                                start=(k == 0), stop=(k == 2 * EC - 1))
                            k += 1
                    nc.scalar.copy(pf_sb[:, h], ps_p)
                    if h >= 1:
                        emit_normT(h - 1)
                emit_normT(HC - 1)

                # ---- per-c tail: q, d2, newton-sqrt, softmax ----
                junk_sb = work.tile([P, H], dt.float32, tag="junk")
                normp_sb = work.tile([P, CC], dt.float32, tag="normp")
                d2_sb = work.tile([P, CC, O], dt.float32, tag="d2")
                s0_sb = work.tile([P, CC, O], dt.float32, tag="s0")
                r_sb = work.tile([P, CC, O], dt.float32, tag="r")
                u_sb = work.tile([P, CC, O], dt.float32, tag="u")
                e_sb = work.tile([P, CC, O], dt.float32, tag="e")
                ssum_sb = work.tile([P, CC], dt.float32, tag="ssum")
                rec_sb = work.tile([P, CC], dt.float32, tag="rec")
                p1_sb = work.tile([P, CC, O], dt.float32, tag="p1")
                msk_sb = work.tile([P, CC, O], dt.float32, tag="msk")

                for c in range(CC):
                    nc.scalar.activation(
                        junk_sb, ps_ns[c],
                        mybir.ActivationFunctionType.Square,
                        accum_out=normp_sb[:, c:c + 1])
                    nc.vector.tensor_tensor(
                        d2_sb[:, c], ps_t[:, c, :O],
                        n128_sb, mybir.AluOpType.add)
                    nc.vector.tensor_tensor(
                        d2_sb[:, c], d2_sb[:, c],
                        ps_t[:, c, O:], mybir.AluOpType.add)
                    nc.vector.tensor_tensor(
                        d2_sb[:, c], d2_sb[:, c],
                        normp_sb[:, c:c + 1].to_broadcast((P, O)),
                        mybir.AluOpType.add)
                    # s0 ~ sqrt(d2) from tables
                    nc.scalar.activation(s0_sb[:, c], d2_sb[:, c],
                                         mybir.ActivationFunctionType.Ln)
                    nc.scalar.activation(s0_sb[:, c], s0_sb[:, c],
                                         mybir.ActivationFunctionType.Exp,
                                         scale=0.5)
                    # newton: u = s0 + d2/s0; e = exp(-u/2)
                    nc.vector.reciprocal(r_sb[:, c], s0_sb[:, c])
                    nc.vector.tensor_tensor(u_sb[:, c], d2_sb[:, c],
                                            r_sb[:, c],
                                            mybir.AluOpType.mult)
                    nc.vector.tensor_tensor(u_sb[:, c], u_sb[:, c],
                                            s0_sb[:, c],
                                            mybir.AluOpType.add)
                    nc.scalar.activation(e_sb[:, c], u_sb[:, c],
                                         mybir.ActivationFunctionType.Exp,
                                         scale=-0.5)
                    nc.vector.reduce_sum(ssum_sb[:, c:c + 1], e_sb[:, c],
                                         axis=mybir.AxisListType.X)
                    nc.vector.reciprocal(rec_sb[:, c:c + 1],
                                         ssum_sb[:, c:c + 1])
                    nc.vector.tensor_tensor(
                        p1_sb[:, c], e_sb[:, c],
                        rec_sb[:, c:c + 1].to_broadcast((P, O)),
                        mybir.AluOpType.mult)
                    nc.vector.tensor_scalar(
                        msk_sb[:, c], p1_sb[:, c], THRESH, None,
                        mybir.AluOpType.is_ge)
                    nc.vector.tensor_tensor(ot_sb[:, tt, c], p1_sb[:, c],
                                            msk_sb[:, c],
                                            mybir.AluOpType.mult)

            nc.sync.dma_start(out_d, ot_sb)

    import concourse.bacc as _bacc_mod
    _orig_gat = _bacc_mod.get_activation_tables
    _KEEP = "natural_log_exp_and_others"

    def _patched_gat(arch):
        return {k: (v if k == _KEEP else set())
                for k, v in _orig_gat(arch).items()}

    _bacc_mod.get_activation_tables = _patched_gat
    try:
        nc.compile()
    finally:
        _bacc_mod.get_activation_tables = _orig_gat
    return nc


def _prep_inputs(word_output, W, b, op_emb):
    x = np.asarray(word_output, np.float32).reshape(TOK, E)
    xh, xl = _split(x)
    xth = xh.T.reshape(EC, P, NCORES, NTT, TT)
    xtl = xl.T.reshape(EC, P, NCORES, NTT, TT)

    Wt64 = np.asarray(W, np.float64).T            # [E, H]
    wh = Wt64.astype(np.float32).astype(BF)       # [E, H] bf16
    wh_t = np.ascontiguousarray(wh.reshape(EC, P, H).transpose(1, 0, 2))

    bd = np.asarray(b, np.float64)
    oed = np.asarray(op_emb, np.float64)
    m = 2.0 * ((bd[None, :] - oed) @ np.asarray(W, np.float64)).T  # [E, O]
    mh, ml = _split(m.astype(np.float32))
    mpk = np.concatenate([mh, ml], axis=1)        # [E, 64]
    mpk_t = np.ascontiguousarray(
        mpk.reshape(EC, P, 2 * O).transpose(1, 0, 2))

    n = ((bd[None, :] - oed) ** 2).sum(-1).astype(np.float32)      # [O]
    n128 = np.broadcast_to(n, (P, O)).copy()

    common = {"wh": wh_t, "mpk": mpk_t, "n128": n128}
    in_maps = []
    for core in range(NCORES):
        mcore = dict(common)
        mcore["xh"] = np.ascontiguousarray(
            xth[:, :, core].transpose(2, 1, 0, 3))
        mcore["xl"] = np.ascontiguousarray(
            xtl[:, :, core].transpose(2, 1, 0, 3))
        in_maps.append(mcore)
    return in_maps


def kernel(word_output, W, b, op_emb, _trace=False):
    if "nc" not in _CACHE:
        _CACHE["nc"] = _build()
    nc = _CACHE["nc"]
    in_maps = _prep_inputs(word_output, W, b, op_emb)
    try:
        res = bass_utils.run_bass_kernel_spmd(
            nc, in_maps, core_ids=list(range(NCORES)), trace=_trace)
    except ModuleNotFoundError:
        res = bass_utils.run_bass_kernel_spmd(
            nc, in_maps, core_ids=list(range(NCORES)), trace=False)
    outs = []
    for r in res.results:
        o = r["out"]                       # [P, NTT, CC, O]
        outs.append(o.transpose(1, 2, 0, 3).reshape(TPC, O))
    out = np.concatenate(outs, axis=0)
    _CACHE["last_results"] = res
    return out.reshape(B, S, O)


if __name__ == "__main__":
    rng = np.random.default_rng(0)
    wo = rng.standard_normal((B, S, E)).astype(np.float32)
    W_ = (rng.standard_normal((H, E)) / np.sqrt(E)).astype(np.float32)
    b_ = (rng.standard_normal(H) * 0.01).astype(np.float32)
    oe = rng.standard_normal((O, H)).astype(np.float32)
    out = kernel(wo, W_, b_, oe)
    proj = wo.reshape(-1, E).astype(np.float64) @ W_.T.astype(np.float64) + b_
    diff = proj[:, None, :] - oe
    d2 = (diff * diff).sum(-1)
    dis = -np.sqrt(d2)
    ex = np.exp(dis - dis.max(-1, keepdims=True))
    prob = ex / ex.sum(-1, keepdims=True)
    ref = np.where(prob < THRESH, 0, prob).astype(np.float32).reshape(B, S, O)
    num = np.linalg.norm(out - ref)
    den = np.linalg.norm(ref)
    print("norm rel err:", num / den)
    print("max abs err:", np.abs(out - ref).max())


# revision 23
# speedup vs baseline: 2.0383x; 1.0062x over previous
"""TRN2 Bass kernel for nn_Knowledge_Base (retrieval_knn).

reference:
    proj = word_output @ W.T + b            # [B,S,H]
    dis  = -sqrt(sum((proj[...,None,:] - op_emb)**2, -1))   # [B,S,O]
    prob = softmax(dis, -1); prob[prob < 0.3] = 0

Strategy (8 cores, data-parallel over the 8192 tokens, 1024/core):
  d2(o) = ||W x||^2 + m_o . x + n_o   with host-precomputed (fp64)
      m_o = 2 W^T (b - e_o)  in R^768,   n_o = ||b - e_o||^2.
  This decouples per-o accuracy (exact m/n + 3-term bf16-split x.m
  matmul) from ||Wx||^2, whose error is shared across all 32 ops and
  attenuated ~60x by the sqrt curvature, so a 2-pass bf16 proj
  (Wh@xh + Wh@xl) suffices; the residual is an algebraic omission
  (missing Wl@x), identical on HW and in simulation (~5e-6 prob error
  at the tightest threshold margin, 0 threshold flips).
  ||Wx||^2: transpose proj via PE identity matmuls into [t, h] PSUM,
  ACT Square + accum row-sum.  4 t-tiles of 256 tokens pipeline the
  per-tile softmax tail under the next tile's matmuls, so only the
  last 256-token tail is exposed.
  sqrt: s0 = exp(0.5 ln d2) from the ACT table, then one DVE Newton
  step u = s0 + d2/s0 (bit-exact reciprocal), final ACT exp(-0.5 u).
  The Newton step removes the ~1e-4 act-table error that otherwise
  flips threshold-marginal probs.  All ACT funcs (copy, square, ln,
  exp) are forced into the one covering act table set, so there are
  no mid-kernel table reloads.
  Host pre/post: x pre-split bf16 hi/lo and pre-tiled so every DMA is
  contiguous per partition (first tile in halves so the PE starts
  early); output written in on-chip [p, c, o] layout, host unpermutes.
"""
import sys
sys.path.insert(0, "/opt/trn_rl_repo")
import numpy as np
import ml_dtypes

import concourse.bacc as bacc
import concourse.tile as tile
from concourse import mybir
from concourse import bass_utils

BF = ml_dtypes.bfloat16
P = 128
B, S, E, H, O = 4, 2048, 768, 512, 32
NCORES = 8
TOK = B * S                  # 8192
TPC = TOK // NCORES          # 1024 tokens per core
TT = 256                     # t-tile size
NTT = TPC // TT              # 4 t-tiles per core
EC = E // P                  # 6 e-chunks
HC = H // P                  # 4 h-chunks
CC = TT // P                 # 2 c-chunks (token blocks) per t-tile
EH = 2                       # e-chunks in first DMA half
THRESH = 0.3

_CACHE = {}


def _split(a):
    hi = a.astype(BF)
    lo = (a.astype(np.float32) - hi.astype(np.float32)).astype(BF)
    return hi, lo


def _build():
    nc = bacc.Bacc("TRN2", target_bir_lowering=False, debug=False,
                   num_devices=NCORES)
    dt = mybir.dt
    xh_d = nc.dram_tensor("xh", [NTT, P, EC, TT], dt.bfloat16,
                          kind="ExternalInput").ap()
    xl_d = nc.dram_tensor("xl", [NTT, P, EC, TT], dt.bfloat16,
                          kind="ExternalInput").ap()
    wh_d = nc.dram_tensor("wh", [P, EC, H], dt.bfloat16,
                          kind="ExternalInput").ap()
    mpk_d = nc.dram_tensor("mpk", [P, EC, 2 * O], dt.bfloat16,
                           kind="ExternalInput").ap()
    n128_d = nc.dram_tensor("n128", [P, O], dt.float32,
                            kind="ExternalInput").ap()
    out_d = nc.dram_tensor("out", [P, NTT, CC, O], dt.float32,
                           kind="ExternalOutput").ap()

    with tile.TileContext(nc) as tc:
        with tc.tile_pool(name="consts", bufs=1) as consts, \
             tc.tile_pool(name="xin", bufs=2) as xin, \
             tc.tile_pool(name="proj", bufs=2) as proj, \
             tc.tile_pool(name="work", bufs=2) as work, \
             tc.tile_pool(name="psp", bufs=2, space="PSUM") as psp, \
             tc.tile_pool(name="psl", bufs=2, space="PSUM") as psl, \
             tc.tile_pool(name="psn", bufs=1, space="PSUM") as psn, \
             tc.tile_pool(name="pst", bufs=2, space="PSUM") as pst:

            mpk_sb = consts.tile([P, EC, 2 * O], dt.bfloat16)
            nc.sync.dma_start(mpk_sb, mpk_d)
            n128_sb = consts.tile([P, O], dt.float32)
            nc.sync.dma_start(n128_sb, n128_d)

            # first tile's x (in halves) ahead of the weights: lin
            # matmuls warm the PE while wh streams in
            xh0_sb = xin.tile([P, EC, TT], dt.bfloat16, tag="xh")
            xl0_sb = xin.tile([P, EC, TT], dt.bfloat16, tag="xl")
            nc.sync.dma_start(xh0_sb[:, :EH], xh_d[0, :, :EH])
            nc.sync.dma_start(xl0_sb[:, :EH], xl_d[0, :, :EH])
            wh_sb = consts.tile([P, EC, H], dt.bfloat16)
            nc.sync.dma_start(wh_sb, wh_d)
            nc.sync.dma_start(xh0_sb[:, EH:], xh_d[0, :, EH:])
            nc.sync.dma_start(xl0_sb[:, EH:], xl_d[0, :, EH:])

            from concourse.masks import make_identity
            ident_sb = consts.tile([P, P], dt.float32)
            make_identity(nc, ident_sb)

            ot_sb = consts.tile([P, NTT, CC, O], dt.float32)

            for tt in range(NTT):
                if tt == 0:
                    xh_sb, xl_sb = xh0_sb, xl0_sb
                else:
                    xh_sb = xin.tile([P, EC, TT], dt.bfloat16, tag="xh")
                    xl_sb = xin.tile([P, EC, TT], dt.bfloat16, tag="xl")
                    nc.sync.dma_start(xh_sb, xh_d[tt])
                    nc.sync.dma_start(xl_sb, xl_d[tt])

                # ---- lin: m_o . x  (3-term split) in [o, t]; the
                # [mh|ml] pair rides one M=64 stationary so xh streams
                # once; rows O:2O hold the xh.ml half, merged after the
                # transpose.  xl.mh passes target rows 0:O only; the
                # last M=64 pass carries the group stop. ----
                ps_l = psl.tile([2 * O, TT], dt.float32, tag="lin")
                nc.tensor.matmul(ps_l, mpk_sb[:, 0, :], xh_sb[:, 0],
                                 start=True, stop=False)
                for e in range(EC):
                    nc.tensor.matmul(
                        ps_l[:O], mpk_sb[:, e, :O], xl_sb[:, e],
                        start=False, stop=False, skip_group_check=True)
                for e in range(1, EC):
                    nc.tensor.matmul(ps_l, mpk_sb[:, e, :], xh_sb[:, e],
                                     start=False, stop=(e == EC - 1),
                                     skip_group_check=True)
                lin_sb = work.tile([2 * O, TT], dt.float32, tag="lin")
                nc.scalar.copy(lin_sb, ps_l)

                # ---- linT: [o, t] -> [t, o] ----
                ps_t = pst.tile([P, CC, 2 * O], dt.float32, tag="linT")
                for c in range(CC):
                    nc.tensor.matmul(
                        ps_t[:, c], lin_sb[:, c * P:(c + 1) * P],
                        ident_sb[:2 * O, :2 * O], is_transpose=True,
                        start=True, stop=True)

                # ---- proj (W x)^T in [h, t], 2-pass, with the norm
                # transposes software-pipelined one h-stage behind ----
                pf_sb = proj.tile([P, HC, TT], dt.float32, tag="pf")
                ps_ns = [psn.tile([P, H], dt.float32, tag=f"pn{c}",
                                  name=f"ps_n{c}")
                         for c in range(CC)]

                def emit_normT(h):
                    hosl = slice(h * P, (h + 1) * P)
                    for c in range(CC):
                        nc.tensor.matmul(
                            ps_ns[c][:, hosl],
                            pf_sb[:, h, c * P:(c + 1) * P], ident_sb,
                            is_transpose=True, start=True,
                            stop=(h == HC - 1))

                for h in range(HC):
                    ps_p = psp.tile([P, TT], dt.float32, tag="pp")
                    hsl = slice(h * P, (h + 1) * P)
                    k = 0
                    for e in range(EC):
                        for xq in (xh_sb, xl_sb):
                            nc.tensor.matmul(
                                ps_p, wh_sb[:, e, hsl], xq[:, e],
                                start=(k == 0), stop=(k == 2 * EC - 1))
                            k += 1
                    nc.scalar.copy(pf_sb[:, h], ps_p)
                    if h >= 1:
                        emit_normT(h - 1)
                emit_normT(HC - 1)

                # ---- per-c tail: q, d2, newton-sqrt, softmax ----
                junk_sb = work.tile([P, H], dt.float32, tag="junk")
                normp_sb = work.tile([P, CC], dt.float32, tag="normp")
                d2_sb = work.tile([P, CC, O], dt.float32, tag="d2")
                s0_sb = work.tile([P, CC, O], dt.float32, tag="s0")
                r_sb = work.tile([P, CC, O], dt.float32, tag="r")
                u_sb = work.tile([P, CC, O], dt.float32, tag="u")
                e_sb = work.tile([P, CC, O], dt.float32, tag="e")
                ssum_sb = work.tile([P, CC], dt.float32, tag="ssum")
                rec_sb = work.tile([P, CC], dt.float32, tag="rec")
                p1_sb = work.tile([P, CC, O], dt.float32, tag="p1")
                msk_sb = work.tile([P, CC, O], dt.float32, tag="msk")

                for c in range(CC):
                    nc.scalar.activation(
                        junk_sb, ps_ns[c],
                        mybir.ActivationFunctionType.Square,
                        accum_out=normp_sb[:, c:c + 1])
                    nc.vector.tensor_tensor(
                        d2_sb[:, c], ps_t[:, c, :O],
                        n128_sb, mybir.AluOpType.add)
                    nc.vector.tensor_tensor(
                        d2_sb[:, c], d2_sb[:, c],
                        ps_t[:, c, O:], mybir.AluOpType.add)
                    nc.vector.tensor_tensor(
                        d2_sb[:, c], d2_sb[:, c],
                        normp_sb[:, c:c + 1].to_broadcast((P, O)),
                        mybir.AluOpType.add)
                    # s0 ~ sqrt(d2) from tables
                    nc.scalar.activation(s0_sb[:, c], d2_sb[:, c],
                                         mybir.ActivationFunctionType.Ln)
                    nc.scalar.activation(s0_sb[:, c], s0_sb[:, c],
                                         mybir.ActivationFunctionType.Exp,
                                         scale=0.5)
                    # newton: u = s0 + d2/s0; e = exp(-u/2)
                    nc.vector.reciprocal(r_sb[:, c], s0_sb[:, c])
                    nc.vector.tensor_tensor(u_sb[:, c], d2_sb[:, c],
                                            r_sb[:, c],
                                            mybir.AluOpType.mult)
                    nc.vector.tensor_tensor(u_sb[:, c], u_sb[:, c],
                                            s0_sb[:, c],
                                            mybir.AluOpType.add)
                    nc.scalar.activation(e_sb[:, c], u_sb[:, c],
                                         mybir.ActivationFunctionType.Exp,
                                         scale=-0.5)
                    nc.vector.reduce_sum(ssum_sb[:, c:c + 1], e_sb[:, c],
                                         axis=mybir.AxisListType.X)
                    nc.vector.reciprocal(rec_sb[:, c:c + 1],
                                         ssum_sb[:, c:c + 1])
                    nc.scalar.activation(
                        p1_sb[:, c], e_sb[:, c],
                        mybir.ActivationFunctionType.Copy,
                        scale=rec_sb[:, c:c + 1])
                    nc.vector.tensor_scalar(
                        msk_sb[:, c], p1_sb[:, c], THRESH, None,
                        mybir.AluOpType.is_ge)
                    nc.vector.tensor_tensor(ot_sb[:, tt, c], p1_sb[:, c],
                                            msk_sb[:, c],
                                            mybir.AluOpType.mult)

            nc.sync.dma_start(out_d, ot_sb)

    import concourse.bacc as _bacc_mod
    _orig_gat = _bacc_mod.get_activation_tables
    _KEEP = "natural_log_exp_and_others"

    def _patched_gat(arch):
        return {k: (v if k == _KEEP else set())
                for k, v in _orig_gat(arch).items()}

    _bacc_mod.get_activation_tables = _patched_gat
    try:
        nc.compile()
    finally:
        _bacc_mod.get_activation_tables = _orig_gat
    return nc


def _prep_inputs(word_output, W, b, op_emb):
    x = np.asarray(word_output, np.float32).reshape(TOK, E)
    xh, xl = _split(x)
    xth = xh.T.reshape(EC, P, NCORES, NTT, TT)
    xtl = xl.T.reshape(EC, P, NCORES, NTT, TT)

    Wt64 = np.asarray(W, np.float64).T            # [E, H]
    wh = Wt64.astype(np.float32).astype(BF)       # [E, H] bf16
    wh_t = np.ascontiguousarray(wh.reshape(EC, P, H).transpose(1, 0, 2))

    bd = np.asarray(b, np.float64)
    oed = np.asarray(op_emb, np.float64)
    m = 2.0 * ((bd[None, :] - oed) @ np.asarray(W, np.float64)).T  # [E, O]
    mh, ml = _split(m.astype(np.float32))
    mpk = np.concatenate([mh, ml], axis=1)        # [E, 64]
    mpk_t = np.ascontiguousarray(
        mpk.reshape(EC, P, 2 * O).transpose(1, 0, 2))

    n = ((bd[None, :] - oed) ** 2).sum(-1).astype(np.float32)      # [O]
    n128 = np.broadcast_to(n, (P, O)).copy()

    common = {"wh": wh_t, "mpk": mpk_t, "n128": n128}
    in_maps = []
    for core in range(NCORES):
        mcore = dict(common)
        mcore["xh"] = np.ascontiguousarray(
            xth[:, :, core].transpose(2, 1, 0, 3))
        mcore["xl"] = np.ascontiguousarray(
            xtl[:, :, core].transpose(2, 1, 0, 3))
        in_maps.append(mcore)
    return in_maps


def kernel(word_output, W, b, op_emb, _trace=False):
    if "nc" not in _CACHE:
        _CACHE["nc"] = _build()
    nc = _CACHE["nc"]
    in_maps = _prep_inputs(word_output, W, b, op_emb)
    try:
        res = bass_utils.run_bass_kernel_spmd(
            nc, in_maps, core_ids=list(range(NCORES)), trace=_trace)
    except ModuleNotFoundError:
        res = bass_utils.run_bass_kernel_spmd(
            nc, in_maps, core_ids=list(range(NCORES)), trace=False)
    outs = []
    for r in res.results:
        o = r["out"]                       # [P, NTT, CC, O]
        outs.append(o.transpose(1, 2, 0, 3).reshape(TPC, O))
    out = np.concatenate(outs, axis=0)
    _CACHE["last_results"] = res
    return out.reshape(B, S, O)


if __name__ == "__main__":
    rng = np.random.default_rng(0)
    wo = rng.standard_normal((B, S, E)).astype(np.float32)
    W_ = (rng.standard_normal((H, E)) / np.sqrt(E)).astype(np.float32)
    b_ = (rng.standard_normal(H) * 0.01).astype(np.float32)
    oe = rng.standard_normal((O, H)).astype(np.float32)
    out = kernel(wo, W_, b_, oe)
    proj = wo.reshape(-1, E).astype(np.float64) @ W_.T.astype(np.float64) + b_
    diff = proj[:, None, :] - oe
    d2 = (diff * diff).sum(-1)
    dis = -np.sqrt(d2)
    ex = np.exp(dis - dis.max(-1, keepdims=True))
    prob = ex / ex.sum(-1, keepdims=True)
    ref = np.where(prob < THRESH, 0, prob).astype(np.float32).reshape(B, S, O)
    num = np.linalg.norm(out - ref)
    den = np.linalg.norm(ref)
    print("norm rel err:", num / den)
    print("max abs err:", np.abs(out - ref).max())
